# revision 11
# baseline (speedup 1.0000x reference)
"""Self-contained Trainium2 Bass kernel for nn_MultiHeadAttention_50800873177468.

B=8, T=1024, D=1024, H=16 causal MHA (Whisper-style). Data-parallel over
batch: core c computes batch c. Returns (out, qk) like the reference.

Per-core dataflow (all matmuls f32r or bf16, fp32 PSUM accumulate):
  phase 1: qT = Wq@xT + bq, kT = 0.125*(Wk@xT), v = x@WvT + bv
  phase 2: per (head, tq-tile): s = qT.T@kT (K=64), +mask on diag block,
           qk out <- s (ACT copy), w = exp(s) bf16 + row-sum (ACT accum),
           w *= 1/sum (DVE), wT = PE-transpose(w), wv^T += v.T@wT
  phase 3: out = wv@WoT + bo
Host: input transposes, -inf fill of the qk upper triangle, stacking.
"""
import os
import sys
import types

for _p in ("/opt/trn_rl_repo", "/root/.axon_site/_ro/trn_rl_repo"):
    if os.path.isdir(_p) and _p not in sys.path:
        sys.path.append(_p)

import numpy as np

# NTFF profile hook shim (missing antenv.axon_hooks in this image)
if 'antenv.axon_hooks' not in sys.modules:
    _m = types.ModuleType('antenv.axon_hooks')
    _h = [None]
    _m.get_axon_ntff_profile_hook = lambda: _h[0]
    _m.set_axon_ntff_profile_hook = lambda h: _h.__setitem__(0, h)
    sys.modules['antenv.axon_hooks'] = _m
    try:
        from trn_agent_boot.trn_boot import _ntff_profile_via_ctypes
        _m.set_axon_ntff_profile_hook(
            _ntff_profile_via_ctypes('/opt/axon/libaxon_pjrt.so'))
    except Exception:
        pass

import concourse.bass as bass
import concourse.tile as tile
from concourse import bacc, mybir
from concourse import bass_utils
from concourse.bass_interp import get_hw_module

f32 = mybir.dt.float32
f32r = mybir.dt.float32r
bf16 = mybir.dt.bfloat16
EXP = mybir.ActivationFunctionType.Exp
ADD = mybir.AluOpType.add

N_CORES = 8
B, T, D, H = 8, 1024, 1024, 16
HD = D // H            # 64
P = 128
NT = T // P            # 8 tq tiles
ND = D // P            # 8 d tiles
S2 = 0.125             # (hd ** -0.25) ** 2, exact in fp32

TRACE = bool(os.environ.get("BASS_KERNEL_TRACE"))
_CACHE = {}


def _emit(nc, tc, ap):
    ctx_pools = []

    def pool(name, **kw):
        p = tc.tile_pool(name=name, **kw)
        return p

    with pool("persist", bufs=1) as persist:
        # small persistent operands
        bqs = persist.tile([P, ND], f32)          # bq as per-partition cols
        nc.sync.dma_start(out=bqs, in_=ap["bq"].rearrange("(g p) -> p g", p=P))
        maskdT = persist.tile([P, P], f32)        # transposed diag mask block
        nc.sync.dma_start(out=maskdT, in_=ap["maskdT"])
        ones_j = persist.tile([1, HD], bf16)      # ones row (broadcast lhsT)
        nc.vector.memset(ones_j, 1.0)
        bvb = persist.tile([P, D], bf16)          # bv broadcast along partitions
        nc.gpsimd.dma_start(out=bvb, in_=bass.AP(
            tensor=ap["bv"].tensor, offset=ap["bv"].offset,
            ap=[[0, P], [1, D]]))
        bob = persist.tile([P, D], f32)           # bo broadcast
        nc.gpsimd.dma_start(out=bob, in_=bass.AP(
            tensor=ap["bo"].tensor, offset=ap["bo"].offset,
            ap=[[0, P], [1, D]]))

        with pool("qkv", bufs=1) as qkv:
            qT = qkv.tile([P, ND, T], f32r)       # q^T  [dout, t]
            kT = qkv.tile([P, ND, T], f32r)       # k^T * 0.125
            vA = qkv.tile([P, NT, H, HD + 1], bf16)  # v | ones column
            wvT = qkv.tile([P, ND, T], bf16)      # (w@v)^T  [dj, tq]

            # ---------------- phase 1: projections ----------------
            with pool("ph1", bufs=1) as ph1, \
                 pool("ph1w", bufs=1) as ph1w, \
                 pool("ps1", bufs=4, space="PSUM") as ps1:
                xT = ph1.tile([P, ND, T], f32r)
                nc.gpsimd.dma_start(
                    out=xT, in_=ap["x"].rearrange("(g p) t -> p g t", p=P))

                for wname, scale in (("WqT", None), ("WkT", S2)):
                    wt = ph1w.tile([P, ND, D], f32r, tag="w")
                    nc.gpsimd.dma_start(
                        out=wt, in_=ap[wname].rearrange("(g p) d -> p g d", p=P))
                    dst = qT if wname == "WqT" else kT
                    for g in range(ND):          # dout tile
                        for c in range(2):       # t chunk of 512
                            ps = ps1.tile([P, 512], f32, tag="ps")
                            for kk in range(ND):
                                nc.tensor.matmul(
                                    ps, wt[:, kk, g * P:(g + 1) * P],
                                    xT[:, kk, c * 512:(c + 1) * 512],
                                    start=(kk == 0), stop=(kk == ND - 1))
                            o = dst[:, g, c * 512:(c + 1) * 512]
                            if scale is None:
                                nc.vector.tensor_scalar_add(o, ps, bqs[:, g:g + 1])
                            else:
                                nc.vector.tensor_scalar_mul(o, ps, scale)

                wt = ph1w.tile([P, ND, D], f32r, tag="w")
                nc.gpsimd.dma_start(
                    out=wt, in_=ap["WvT"].rearrange("(g p) d -> p g d", p=P))
                nc.vector.memset(vA[:, :, :, HD:HD + 1], 1.0)
                for tt in range(NT):
                    for c in range(2):
                        ps = ps1.tile([P, 512], f32, tag="ps")
                        for kk in range(ND):
                            nc.tensor.matmul(
                                ps, xT[:, kk, tt * P:(tt + 1) * P],
                                wt[:, kk, c * 512:(c + 1) * 512],
                                start=(kk == 0), stop=(kk == ND - 1))
                        for hh in range(8 * c, 8 * c + 8):
                            nc.vector.tensor_tensor(
                                out=vA[:, tt, hh, 0:HD],
                                in0=ps[:, (hh - 8 * c) * HD:(hh - 8 * c + 1) * HD],
                                in1=bvb[:, hh * HD:(hh + 1) * HD], op=ADD)

            # ---------------- phase 2: attention ----------------
            # Two score passes, no PE transposes:
            #  pass A [tq,tk] feeds the qk output; pass B [tk,tq] feeds
            #  exp -> w^T (bf16). Row sums via ones-matmul on w^T (l̃ lands
            #  [1,tq] in PSUM), normalization as (ones64 x 1/l̃) rank-1
            #  PSUM tile multiplied into the wv PSUM during copy-out.
            with pool("ph2s", bufs=4) as ph2s, \
                 pool("ph2w", bufs=2) as ph2w, \
                 pool("psA", bufs=2, space="PSUM") as psA, \
                 pool("psB", bufs=3, space="PSUM") as psB, \
                 pool("psW", bufs=2, space="PSUM") as psW, \
                 pool("psR", bufs=1, space="PSUM") as psR:
                for h in range(H):
                    g, ro = h // 2, (h % 2) * HD
                    # --- pass A: scores in [tq, tk] for the qk output ---
                    for i in range(NT):
                        span = (i + 1) * P
                        qkst = ph2s.tile([P, T], f32, tag="qkst")
                        for c0 in range(0, span, 512):
                            c1 = min(c0 + 512, span)
                            sa = psA.tile([P, 512], f32, tag="sA")
                            nc.tensor.matmul(
                                sa[:, :c1 - c0],
                                qT[ro:ro + HD, g, i * P:(i + 1) * P],
                                kT[ro:ro + HD, g, c0:c1],
                                start=True, stop=True)
                            eng = nc.scalar if (i + h) % 2 else nc.vector
                            if eng is nc.scalar:
                                nc.scalar.copy(qkst[:, c0:c1], sa[:, :c1 - c0])
                            else:
                                nc.vector.tensor_copy(qkst[:, c0:c1],
                                                      sa[:, :c1 - c0])
                        nc.sync.dma_start(
                            out=ap["qk"][h, i * P:(i + 1) * P, 0:span],
                            in_=qkst[:, :span])
                    # --- pass B: scores in [tk, tq] -> exp -> w^T bf16 ---
                    wTu = ph2w.tile([P, NT, T], bf16, tag="wTu")
                    for t in range(NT):
                        tq0 = t * P
                        for c in range(2):
                            c0, c1 = max(c * 512, tq0), (c + 1) * 512
                            if c0 >= c1:
                                continue
                            sb = psB.tile([P, 512], f32, tag="sB")
                            nc.tensor.matmul(
                                sb[:, :c1 - c0],
                                kT[ro:ro + HD, g, tq0:(t + 1) * P],
                                qT[ro:ro + HD, g, c0:c1],
                                start=True, stop=True)
                            if c0 == tq0:  # chunk starts at the diag block
                                nc.vector.tensor_tensor(
                                    out=sb[:, 0:P], in0=sb[:, 0:P],
                                    in1=maskdT, op=ADD)
                            nc.scalar.activation(
                                wTu[:, t, c0:c1], sb[:, :c1 - c0], EXP)
                    # --- wv (+ fused l̃ row via the ones column of vA) ---
                    for c in range(2):
                        wps = psW.tile([HD + 1, 512], f32, tag="wv")
                        nmm = 4 * c + 4
                        for t in range(nmm):
                            cs = max(c * 512, t * P)
                            nc.tensor.matmul(
                                wps[:, cs - c * 512:512],
                                vA[:, t, h, :],
                                wTu[:, t, cs:(c + 1) * 512],
                                start=(t == 0), stop=(t == nmm - 1))
                        lt = ph2s.tile([1, 512], f32, tag="lt")
                        nc.vector.tensor_copy(lt, wps[HD:HD + 1, :])
                        rt = ph2s.tile([1, 512], f32, tag="rt")
                        nc.vector.reciprocal_approx_fast(rt, lt)
                        rtb = ph2s.tile([1, 512], bf16, tag="rtb")
                        nc.vector.tensor_copy(rtb, rt)
                        rbc = psR.tile([HD, 512], f32, tag="rbc")
                        nc.tensor.matmul(rbc, ones_j, rtb, start=True, stop=True)
                        rbcs = ph2s.tile([HD, 512], bf16, tag="rbcs")
                        nc.scalar.copy(rbcs, rbc)
                        nc.vector.tensor_tensor(
                            out=wvT[ro:ro + HD, g, c * 512:(c + 1) * 512],
                            in0=wps[0:HD, :], in1=rbcs, op=mybir.AluOpType.mult)

        # ---------------- phase 3: output projection ----------------
        with pool("ph3", bufs=1) as ph3, \
             pool("ph3s", bufs=3) as ph3s, \
             pool("ps3", bufs=4, space="PSUM") as ps3:
            wo = ph3.tile([P, ND, D], bf16)
            nc.gpsimd.dma_start(
                out=wo, in_=ap["WoT"].rearrange("(g p) d -> p g d", p=P))
            for it in range(NT):
                ost = ph3s.tile([P, D], f32, tag="ost")
                for c in range(2):
                    ps = ps3.tile([P, 512], f32, tag="ps")
                    for g in range(ND):
                        nc.tensor.matmul(
                            ps, wvT[:, g, it * P:(it + 1) * P],
                            wo[:, g, c * 512:(c + 1) * 512],
                            start=(g == 0), stop=(g == ND - 1))
                    nc.vector.tensor_tensor(
                        out=ost[:, c * 512:(c + 1) * 512], in0=ps,
                        in1=bob[:, c * 512:(c + 1) * 512], op=ADD)
                nc.sync.dma_start(
                    out=ap["out"][it * P:(it + 1) * P, :], in_=ost)


def _build():
    if "nc" in _CACHE:
        return _CACHE["nc"]
    nc = bacc.Bacc("TRN2", target_bir_lowering=False, debug=False,
                   enable_asserts=False, num_devices=N_CORES)
    ap = {}
    for name, shape in (("x", [D, T]), ("WqT", [D, D]), ("WkT", [D, D]),
                        ("WvT", [D, D]), ("WoT", [D, D]), ("bq", [D]),
                        ("bv", [D]), ("bo", [D]), ("maskdT", [P, P])):
        ap[name] = nc.dram_tensor(name, shape, f32, kind="ExternalInput").ap()
    ap["out"] = nc.dram_tensor("out", [T, D], f32, kind="ExternalOutput").ap()
    ap["qk"] = nc.dram_tensor("qk", [H, T, T], f32, kind="ExternalOutput").ap()

    with tile.TileContext(nc) as tc:
        _emit(nc, tc, ap)
    nc.compile()
    nc.m = get_hw_module(nc.m)
    _CACHE["nc"] = nc
    return nc


def kernel(x, mask, Wq, bq, Wk, Wv, bv, Wo, bo):
    nc = _build()
    x = np.ascontiguousarray(x, dtype=np.float32)
    base = {
        "WqT": np.ascontiguousarray(Wq.T, dtype=np.float32),
        "WkT": np.ascontiguousarray(Wk.T, dtype=np.float32),
        "WvT": np.ascontiguousarray(Wv.T, dtype=np.float32),
        "WoT": np.ascontiguousarray(Wo.T, dtype=np.float32),
        "bq": np.ascontiguousarray(bq, dtype=np.float32),
        "bv": np.ascontiguousarray(bv, dtype=np.float32),
        "bo": np.ascontiguousarray(bo, dtype=np.float32),
        "maskdT": np.ascontiguousarray(mask[:P, :P].T, dtype=np.float32),
    }
    in_maps = [dict(base, x=np.ascontiguousarray(x[c].T)) for c in range(B)]

    res = bass_utils.run_bass_kernel_spmd(
        nc, in_maps, core_ids=list(range(N_CORES)), trace=TRACE)
    if TRACE:
        _CACHE["last_results"] = res

    out = np.stack([res.results[c]["out"] for c in range(B)])
    qk = np.stack([res.results[c]["qk"] for c in range(B)])
    triu = np.triu(np.ones((T, T), dtype=bool), k=1)
    qk[:, :, triu] = -np.inf
    return out, qk


# revision 12
# speedup vs baseline: 1.0553x; 1.0553x over previous
"""Self-contained Trainium2 Bass kernel for nn_MultiHeadAttention_50800873177468.

B=8, T=1024, D=1024, H=16 causal MHA (Whisper-style). Data-parallel over
batch: core c computes batch c. Returns (out, qk) like the reference.

Per-core dataflow (all matmuls f32r or bf16, fp32 PSUM accumulate):
  phase 1: qT = Wq@xT + bq, kT = 0.125*(Wk@xT), v = x@WvT + bv
  phase 2: per (head, tq-tile): s = qT.T@kT (K=64), +mask on diag block,
           qk out <- s (ACT copy), w = exp(s) bf16 + row-sum (ACT accum),
           w *= 1/sum (DVE), wT = PE-transpose(w), wv^T += v.T@wT
  phase 3: out = wv@WoT + bo
Host: input transposes, -inf fill of the qk upper triangle, stacking.
"""
import os
import sys
import types

for _p in ("/opt/trn_rl_repo", "/root/.axon_site/_ro/trn_rl_repo"):
    if os.path.isdir(_p) and _p not in sys.path:
        sys.path.append(_p)

import numpy as np

# NTFF profile hook shim (missing antenv.axon_hooks in this image)
if 'antenv.axon_hooks' not in sys.modules:
    _m = types.ModuleType('antenv.axon_hooks')
    _h = [None]
    _m.get_axon_ntff_profile_hook = lambda: _h[0]
    _m.set_axon_ntff_profile_hook = lambda h: _h.__setitem__(0, h)
    sys.modules['antenv.axon_hooks'] = _m
    try:
        from trn_agent_boot.trn_boot import _ntff_profile_via_ctypes
        _m.set_axon_ntff_profile_hook(
            _ntff_profile_via_ctypes('/opt/axon/libaxon_pjrt.so'))
    except Exception:
        pass

import concourse.bass as bass
import concourse.tile as tile
from concourse import bacc, mybir
from concourse import bass_utils
from concourse.bass_interp import get_hw_module

f32 = mybir.dt.float32
f32r = mybir.dt.float32r
bf16 = mybir.dt.bfloat16
EXP = mybir.ActivationFunctionType.Exp
ADD = mybir.AluOpType.add

N_CORES = 8
B, T, D, H = 8, 1024, 1024, 16
HD = D // H            # 64
P = 128
NT = T // P            # 8 tq tiles
ND = D // P            # 8 d tiles
S2 = 0.125             # (hd ** -0.25) ** 2, exact in fp32

TRACE = bool(os.environ.get("BASS_KERNEL_TRACE"))
_CACHE = {}


def _emit(nc, tc, ap):
    ctx_pools = []

    def pool(name, **kw):
        p = tc.tile_pool(name=name, **kw)
        return p

    with pool("persist", bufs=1) as persist:
        # small persistent operands
        bqs = persist.tile([P, ND], f32)          # bq as per-partition cols
        nc.sync.dma_start(out=bqs, in_=ap["bq"].rearrange("(g p) -> p g", p=P))
        maskdT = persist.tile([P, P], f32)        # transposed diag mask block
        nc.sync.dma_start(out=maskdT, in_=ap["maskdT"])
        ones_j = persist.tile([1, HD], bf16)      # ones row (broadcast lhsT)
        nc.vector.memset(ones_j, 1.0)
        bvb = persist.tile([P, D], bf16)          # bv broadcast along partitions
        nc.gpsimd.dma_start(out=bvb, in_=bass.AP(
            tensor=ap["bv"].tensor, offset=ap["bv"].offset,
            ap=[[0, P], [1, D]]))
        bob = persist.tile([P, D], f32)           # bo broadcast
        nc.gpsimd.dma_start(out=bob, in_=bass.AP(
            tensor=ap["bo"].tensor, offset=ap["bo"].offset,
            ap=[[0, P], [1, D]]))

        with pool("qkv", bufs=1) as qkv:
            qT = qkv.tile([P, ND, T], f32r)       # q^T  [dout, t]
            kT = qkv.tile([P, ND, T], f32r)       # k^T * 0.125
            qTb = qkv.tile([P, ND, T], bf16)      # bf16 copies (softmax path)
            kTb = qkv.tile([P, ND, T], bf16)
            vA = qkv.tile([P, NT, H, HD + 1], bf16)  # v | ones column
            wvT = qkv.tile([P, ND, T], bf16)      # (w@v)^T  [dj, tq]

            # ---------------- phase 1: projections ----------------
            with pool("ph1", bufs=1) as ph1, \
                 pool("ph1w", bufs=1) as ph1w, \
                 pool("ps1", bufs=4, space="PSUM") as ps1:
                xT = ph1.tile([P, ND, T], f32r)
                nc.gpsimd.dma_start(
                    out=xT, in_=ap["x"].rearrange("(g p) t -> p g t", p=P))

                for wname, scale in (("WqT", None), ("WkT", S2)):
                    wt = ph1w.tile([P, ND, D], f32r, tag="w")
                    nc.gpsimd.dma_start(
                        out=wt, in_=ap[wname].rearrange("(g p) d -> p g d", p=P))
                    dst = qT if wname == "WqT" else kT
                    for g in range(ND):          # dout tile
                        for c in range(2):       # t chunk of 512
                            ps = ps1.tile([P, 512], f32, tag="ps")
                            for kk in range(ND):
                                nc.tensor.matmul(
                                    ps, wt[:, kk, g * P:(g + 1) * P],
                                    xT[:, kk, c * 512:(c + 1) * 512],
                                    start=(kk == 0), stop=(kk == ND - 1))
                            o = dst[:, g, c * 512:(c + 1) * 512]
                            dstb = qTb if wname == "WqT" else kTb
                            ob = dstb[:, g, c * 512:(c + 1) * 512]
                            if scale is None:
                                nc.vector.tensor_scalar_add(o, ps, bqs[:, g:g + 1])
                            else:
                                nc.vector.tensor_scalar_mul(o, ps, scale)
                            nc.gpsimd.tensor_copy(ob, o)

                wt = ph1w.tile([P, ND, D], f32r, tag="w")
                nc.gpsimd.dma_start(
                    out=wt, in_=ap["WvT"].rearrange("(g p) d -> p g d", p=P))
                nc.vector.memset(vA[:, :, :, HD:HD + 1], 1.0)
                for tt in range(NT):
                    for c in range(2):
                        ps = ps1.tile([P, 512], f32, tag="ps")
                        for kk in range(ND):
                            nc.tensor.matmul(
                                ps, xT[:, kk, tt * P:(tt + 1) * P],
                                wt[:, kk, c * 512:(c + 1) * 512],
                                start=(kk == 0), stop=(kk == ND - 1))
                        for hh in range(8 * c, 8 * c + 8):
                            nc.vector.tensor_tensor(
                                out=vA[:, tt, hh, 0:HD],
                                in0=ps[:, (hh - 8 * c) * HD:(hh - 8 * c + 1) * HD],
                                in1=bvb[:, hh * HD:(hh + 1) * HD], op=ADD)

            # ---------------- phase 2: attention ----------------
            # Two score passes, no PE transposes:
            #  pass A [tq,tk] feeds the qk output; pass B [tk,tq] feeds
            #  exp -> w^T (bf16). Row sums via ones-matmul on w^T (l̃ lands
            #  [1,tq] in PSUM), normalization as (ones64 x 1/l̃) rank-1
            #  PSUM tile multiplied into the wv PSUM during copy-out.
            with pool("ph2s", bufs=4) as ph2s, \
                 pool("ph2w", bufs=2) as ph2w, \
                 pool("psA", bufs=2, space="PSUM") as psA, \
                 pool("psB", bufs=3, space="PSUM") as psB, \
                 pool("psW", bufs=2, space="PSUM") as psW, \
                 pool("psR", bufs=1, space="PSUM") as psR:
                for h in range(H):
                    g, ro = h // 2, (h % 2) * HD
                    # --- pass A: scores in [tq, tk] for the qk output ---
                    for i in range(NT):
                        span = (i + 1) * P
                        qkst = ph2s.tile([P, T], f32, tag="qkst")
                        for c0 in range(0, span, 512):
                            c1 = min(c0 + 512, span)
                            sa = psA.tile([P, 512], f32, tag="sA")
                            nc.tensor.matmul(
                                sa[:, :c1 - c0],
                                qT[ro:ro + HD, g, i * P:(i + 1) * P],
                                kT[ro:ro + HD, g, c0:c1],
                                start=True, stop=True)
                            eng = nc.scalar if (i + h) % 2 else nc.vector
                            if eng is nc.scalar:
                                nc.scalar.copy(qkst[:, c0:c1], sa[:, :c1 - c0])
                            else:
                                nc.vector.tensor_copy(qkst[:, c0:c1],
                                                      sa[:, :c1 - c0])
                        nc.sync.dma_start(
                            out=ap["qk"][h, i * P:(i + 1) * P, 0:span],
                            in_=qkst[:, :span])
                    # --- pass B: scores in [tk, tq] -> exp -> w^T bf16 ---
                    wTu = ph2w.tile([P, NT, T], bf16, tag="wTu")
                    for t in range(NT):
                        tq0 = t * P
                        for c in range(2):
                            c0, c1 = max(c * 512, tq0), (c + 1) * 512
                            if c0 >= c1:
                                continue
                            sb = psB.tile([P, 512], f32, tag="sB")
                            nc.tensor.matmul(
                                sb[:, :c1 - c0],
                                kTb[ro:ro + HD, g, tq0:(t + 1) * P],
                                qTb[ro:ro + HD, g, c0:c1],
                                start=True, stop=True)
                            if c0 == tq0:  # chunk starts at the diag block
                                nc.vector.tensor_tensor(
                                    out=sb[:, 0:P], in0=sb[:, 0:P],
                                    in1=maskdT, op=ADD)
                            nc.scalar.activation(
                                wTu[:, t, c0:c1], sb[:, :c1 - c0], EXP)
                    # --- wv (+ fused l̃ row via the ones column of vA) ---
                    for c in range(2):
                        wps = psW.tile([HD + 1, 512], f32, tag="wv")
                        nmm = 4 * c + 4
                        for t in range(nmm):
                            cs = max(c * 512, t * P)
                            nc.tensor.matmul(
                                wps[:, cs - c * 512:512],
                                vA[:, t, h, :],
                                wTu[:, t, cs:(c + 1) * 512],
                                start=(t == 0), stop=(t == nmm - 1))
                        lt = ph2s.tile([1, 512], f32, tag="lt")
                        nc.vector.tensor_copy(lt, wps[HD:HD + 1, :])
                        rt = ph2s.tile([1, 512], f32, tag="rt")
                        nc.vector.reciprocal_approx_fast(rt, lt)
                        rtb = ph2s.tile([1, 512], bf16, tag="rtb")
                        nc.vector.tensor_copy(rtb, rt)
                        rbc = psR.tile([HD, 512], f32, tag="rbc")
                        nc.tensor.matmul(rbc, ones_j, rtb, start=True, stop=True)
                        rbcs = ph2s.tile([HD, 512], bf16, tag="rbcs")
                        nc.scalar.copy(rbcs, rbc)
                        nc.vector.tensor_tensor(
                            out=wvT[ro:ro + HD, g, c * 512:(c + 1) * 512],
                            in0=wps[0:HD, :], in1=rbcs, op=mybir.AluOpType.mult)

        # ---------------- phase 3: output projection ----------------
        with pool("ph3", bufs=1) as ph3, \
             pool("ph3s", bufs=3) as ph3s, \
             pool("ps3", bufs=4, space="PSUM") as ps3:
            wo = ph3.tile([P, ND, D], bf16)
            nc.gpsimd.dma_start(
                out=wo, in_=ap["WoT"].rearrange("(g p) d -> p g d", p=P))
            for it in range(NT):
                ost = ph3s.tile([P, D], f32, tag="ost")
                for c in range(2):
                    ps = ps3.tile([P, 512], f32, tag="ps")
                    for g in range(ND):
                        nc.tensor.matmul(
                            ps, wvT[:, g, it * P:(it + 1) * P],
                            wo[:, g, c * 512:(c + 1) * 512],
                            start=(g == 0), stop=(g == ND - 1))
                    nc.vector.tensor_tensor(
                        out=ost[:, c * 512:(c + 1) * 512], in0=ps,
                        in1=bob[:, c * 512:(c + 1) * 512], op=ADD)
                nc.sync.dma_start(
                    out=ap["out"][it * P:(it + 1) * P, :], in_=ost)


def _build():
    if "nc" in _CACHE:
        return _CACHE["nc"]
    nc = bacc.Bacc("TRN2", target_bir_lowering=False, debug=False,
                   enable_asserts=False, num_devices=N_CORES)
    ap = {}
    for name, shape in (("x", [D, T]), ("WqT", [D, D]), ("WkT", [D, D]),
                        ("WvT", [D, D]), ("WoT", [D, D]), ("bq", [D]),
                        ("bv", [D]), ("bo", [D]), ("maskdT", [P, P])):
        ap[name] = nc.dram_tensor(name, shape, f32, kind="ExternalInput").ap()
    ap["out"] = nc.dram_tensor("out", [T, D], f32, kind="ExternalOutput").ap()
    ap["qk"] = nc.dram_tensor("qk", [H, T, T], f32, kind="ExternalOutput").ap()

    with tile.TileContext(nc) as tc:
        _emit(nc, tc, ap)
    nc.compile()
    nc.m = get_hw_module(nc.m)
    _CACHE["nc"] = nc
    return nc


def kernel(x, mask, Wq, bq, Wk, Wv, bv, Wo, bo):
    nc = _build()
    x = np.ascontiguousarray(x, dtype=np.float32)
    base = {
        "WqT": np.ascontiguousarray(Wq.T, dtype=np.float32),
        "WkT": np.ascontiguousarray(Wk.T, dtype=np.float32),
        "WvT": np.ascontiguousarray(Wv.T, dtype=np.float32),
        "WoT": np.ascontiguousarray(Wo.T, dtype=np.float32),
        "bq": np.ascontiguousarray(bq, dtype=np.float32),
        "bv": np.ascontiguousarray(bv, dtype=np.float32),
        "bo": np.ascontiguousarray(bo, dtype=np.float32),
        "maskdT": np.ascontiguousarray(mask[:P, :P].T, dtype=np.float32),
    }
    in_maps = [dict(base, x=np.ascontiguousarray(x[c].T)) for c in range(B)]

    res = bass_utils.run_bass_kernel_spmd(
        nc, in_maps, core_ids=list(range(N_CORES)), trace=TRACE)
    if TRACE:
        _CACHE["last_results"] = res

    out = np.stack([res.results[c]["out"] for c in range(B)])
    qk = np.stack([res.results[c]["qk"] for c in range(B)])
    triu = np.triu(np.ones((T, T), dtype=bool), k=1)
    qk[:, :, triu] = -np.inf
    return out, qk


# revision 14
# speedup vs baseline: 1.1564x; 1.0958x over previous
"""Self-contained Trainium2 Bass kernel for nn_MultiHeadAttention_50800873177468.

B=8, T=1024, D=1024, H=16 causal MHA (Whisper-style). Data-parallel over
batch: core c computes batch c. Returns (out, qk) like the reference.

Per-core dataflow (all matmuls f32r or bf16, fp32 PSUM accumulate):
  phase 1: qT = Wq@xT + bq, kT = 0.125*(Wk@xT), v = x@WvT + bv
  phase 2: per (head, tq-tile): s = qT.T@kT (K=64), +mask on diag block,
           qk out <- s (ACT copy), w = exp(s) bf16 + row-sum (ACT accum),
           w *= 1/sum (DVE), wT = PE-transpose(w), wv^T += v.T@wT
  phase 3: out = wv@WoT + bo
Host: input transposes, -inf fill of the qk upper triangle, stacking.
"""
import os
import sys
import types

for _p in ("/opt/trn_rl_repo", "/root/.axon_site/_ro/trn_rl_repo"):
    if os.path.isdir(_p) and _p not in sys.path:
        sys.path.append(_p)

import numpy as np

# NTFF profile hook shim (missing antenv.axon_hooks in this image)
if 'antenv.axon_hooks' not in sys.modules:
    _m = types.ModuleType('antenv.axon_hooks')
    _h = [None]
    _m.get_axon_ntff_profile_hook = lambda: _h[0]
    _m.set_axon_ntff_profile_hook = lambda h: _h.__setitem__(0, h)
    sys.modules['antenv.axon_hooks'] = _m
    try:
        from trn_agent_boot.trn_boot import _ntff_profile_via_ctypes
        _m.set_axon_ntff_profile_hook(
            _ntff_profile_via_ctypes('/opt/axon/libaxon_pjrt.so'))
    except Exception:
        pass

import concourse.bass as bass
import concourse.tile as tile
from concourse import bacc, mybir
from concourse import bass_utils
from concourse.bass_interp import get_hw_module

f32 = mybir.dt.float32
f32r = mybir.dt.float32r
bf16 = mybir.dt.bfloat16
EXP = mybir.ActivationFunctionType.Exp
ADD = mybir.AluOpType.add

N_CORES = 8
B, T, D, H = 8, 1024, 1024, 16
HD = D // H            # 64
P = 128
NT = T // P            # 8 tq tiles
ND = D // P            # 8 d tiles
S2 = 0.125             # (hd ** -0.25) ** 2, exact in fp32

TRACE = bool(os.environ.get("BASS_KERNEL_TRACE"))
_CACHE = {}


def _emit(nc, tc, ap):
    ctx_pools = []

    def pool(name, **kw):
        p = tc.tile_pool(name=name, **kw)
        return p

    with pool("persist", bufs=1) as persist:
        # small persistent operands
        bqs = persist.tile([P, ND], f32)          # bq as per-partition cols
        nc.sync.dma_start(out=bqs, in_=ap["bq"].rearrange("(g p) -> p g", p=P))
        maskdT = persist.tile([P, P], f32)        # transposed diag mask block
        nc.sync.dma_start(out=maskdT, in_=ap["maskdT"])
        ones_j = persist.tile([1, HD], bf16)      # ones row (broadcast lhsT)
        nc.vector.memset(ones_j, 1.0)
        bvb = persist.tile([P, D], bf16)          # bv broadcast along partitions
        nc.gpsimd.dma_start(out=bvb, in_=bass.AP(
            tensor=ap["bv"].tensor, offset=ap["bv"].offset,
            ap=[[0, P], [1, D]]))
        bob = persist.tile([P, D], f32)           # bo broadcast
        nc.gpsimd.dma_start(out=bob, in_=bass.AP(
            tensor=ap["bo"].tensor, offset=ap["bo"].offset,
            ap=[[0, P], [1, D]]))

        with pool("qkv", bufs=1) as qkv:
            qT = qkv.tile([P, ND, T], f32r)       # q^T  [dout, t]
            kT = qkv.tile([P, ND, T], f32r)       # k^T * 0.125
            qTb = qkv.tile([P, ND, T], bf16)      # bf16 copies (softmax path)
            kTb = qkv.tile([P, ND, T], bf16)
            vA = qkv.tile([P, NT, H, HD + 1], bf16)  # v | ones column
            wvT = qkv.tile([P, ND, T], bf16)      # (w@v)^T  [dj, tq]

            # ---------------- phase 1: projections ----------------
            # kk-outer with 8 live PSUM banks per half; weights stream in
            # per-k-tile chunks so DMA pipelines under the matmuls and the
            # PE never stalls at matrix boundaries (HAM stays warm).
            with pool("ph1", bufs=1) as ph1, \
                 pool("ph1w", bufs=6) as ph1w, \
                 pool("ps1", bufs=8, space="PSUM") as ps1:
                xT = ph1.tile([P, ND, T], f32r)
                for kk in range(ND):
                    nc.gpsimd.dma_start(
                        out=xT[:, kk, :], in_=ap["x"][kk * P:(kk + 1) * P, :])
                nc.vector.memset(vA[:, :, :, HD:HD + 1], 1.0)

                for wname in ("WqT", "WkT", "WvT"):
                    for half in range(2):
                        pss = [ps1.tile([P, 512], f32, tag="ps",
                                        name=f"ps_{wname}_{half}_{j}")
                               for j in range(8)]
                        for kk in range(ND):
                            wtk = ph1w.tile([P, D], f32r, tag="w",
                                            name=f"w_{wname}_{half}_{kk}")
                            nc.gpsimd.dma_start(
                                out=wtk, in_=ap[wname][kk * P:(kk + 1) * P, :])
                            for j in range(8):
                                g, c = 4 * half + j // 2, j % 2
                                if wname == "WvT":
                                    lhsT = xT[:, kk, g * P:(g + 1) * P]
                                    rhs = wtk[:, c * 512:(c + 1) * 512]
                                else:
                                    lhsT = wtk[:, g * P:(g + 1) * P]
                                    rhs = xT[:, kk, c * 512:(c + 1) * 512]
                                nc.tensor.matmul(
                                    pss[j], lhsT, rhs,
                                    start=(kk == 0), stop=(kk == ND - 1))
                        for j in range(8):
                            g, c = 4 * half + j // 2, j % 2
                            ps = pss[j]
                            if wname == "WqT":
                                o = qT[:, g, c * 512:(c + 1) * 512]
                                nc.vector.tensor_scalar_add(o, ps, bqs[:, g:g + 1])
                                nc.gpsimd.tensor_copy(
                                    qTb[:, g, c * 512:(c + 1) * 512], o)
                            elif wname == "WkT":
                                o = kT[:, g, c * 512:(c + 1) * 512]
                                nc.vector.tensor_scalar_mul(o, ps, S2)
                                nc.gpsimd.tensor_copy(
                                    kTb[:, g, c * 512:(c + 1) * 512], o)
                            else:
                                tt = g  # [t-tile, d-chunk] output for v
                                for hh in range(8 * c, 8 * c + 8):
                                    nc.vector.tensor_tensor(
                                        out=vA[:, tt, hh, 0:HD],
                                        in0=ps[:, (hh - 8 * c) * HD:
                                               (hh - 8 * c + 1) * HD],
                                        in1=bvb[:, hh * HD:(hh + 1) * HD],
                                        op=ADD)

            # ---------------- phase 2: attention ----------------
            # Two score passes, no PE transposes:
            #  pass A [tq,tk] feeds the qk output; pass B [tk,tq] feeds
            #  exp -> w^T (bf16). Row sums via ones-matmul on w^T (l̃ lands
            #  [1,tq] in PSUM), normalization as (ones64 x 1/l̃) rank-1
            #  PSUM tile multiplied into the wv PSUM during copy-out.
            with pool("ph2s", bufs=4) as ph2s, \
                 pool("ph2w", bufs=2) as ph2w, \
                 pool("psA", bufs=2, space="PSUM") as psA, \
                 pool("psB", bufs=2, space="PSUM") as psB, \
                 pool("psW", bufs=2, space="PSUM") as psW:
                for h in range(H):
                    g, ro = h // 2, (h % 2) * HD
                    # --- pass A: scores in [tq, tk] for the qk output ---
                    for i in range(NT):
                        span = (i + 1) * P
                        qkst = ph2s.tile([P, T], f32, tag="qkst")
                        for c0 in range(0, span, 512):
                            c1 = min(c0 + 512, span)
                            sa = psA.tile([P, 512], f32, tag="sA")
                            nc.tensor.matmul(
                                sa[:, :c1 - c0],
                                qT[ro:ro + HD, g, i * P:(i + 1) * P],
                                kT[ro:ro + HD, g, c0:c1],
                                start=True, stop=True)
                            if (i * H + h) % 3 == 2:
                                nc.scalar.copy(qkst[:, c0:c1], sa[:, :c1 - c0])
                            else:
                                nc.vector.tensor_copy(qkst[:, c0:c1],
                                                      sa[:, :c1 - c0])
                        nc.sync.dma_start(
                            out=ap["qk"][h, i * P:(i + 1) * P, 0:span],
                            in_=qkst[:, :span])
                    # --- pass B: scores in [tk, tq] -> exp -> w^T bf16 ---
                    wTu = ph2w.tile([P, NT, T], bf16, tag="wTu")
                    for t in range(NT):
                        tq0 = t * P
                        width = T - tq0
                        sb = psB.tile([P, T], f32, tag="sB")
                        for c0 in range(tq0, T, 512):
                            c1 = min(c0 + 512, T)
                            nc.tensor.matmul(
                                sb[:, c0 - tq0:c1 - tq0],
                                kTb[ro:ro + HD, g, tq0:(t + 1) * P],
                                qTb[ro:ro + HD, g, c0:c1],
                                start=True, stop=True)
                        nc.vector.tensor_tensor(
                            out=sb[:, 0:P], in0=sb[:, 0:P],
                            in1=maskdT, op=ADD)
                        nc.scalar.activation(
                            wTu[:, t, tq0:T], sb[:, :width], EXP)
                    # --- wv (+ fused l̃ row via the ones column of vA) ---
                    for c in range(2):
                        wps = psW.tile([HD + 1, 512], f32, tag="wv")
                        nmm = 4 * c + 4
                        for t in range(nmm):
                            cs = max(c * 512, t * P)
                            nc.tensor.matmul(
                                wps[:, cs - c * 512:512],
                                vA[:, t, h, :],
                                wTu[:, t, cs:(c + 1) * 512],
                                start=(t == 0), stop=(t == nmm - 1))
                        lt = ph2s.tile([1, 512], f32, tag="lt")
                        nc.vector.tensor_copy(lt, wps[HD:HD + 1, :])
                        rt = ph2s.tile([1, 512], f32, tag="rt")
                        nc.vector.reciprocal_approx_fast(rt, lt)
                        rtb = ph2s.tile([1, 512], bf16, tag="rtb")
                        nc.vector.tensor_copy(rtb, rt)
                        rbc = psA.tile([P, 512], f32, tag="sA",
                                       name=f"rbc{h}_{c}")[0:HD, :]
                        nc.tensor.matmul(rbc, ones_j, rtb, start=True, stop=True)
                        rbcs = ph2s.tile([HD, 512], bf16, tag="rbcs")
                        nc.scalar.copy(rbcs, rbc)
                        nc.vector.tensor_tensor(
                            out=wvT[ro:ro + HD, g, c * 512:(c + 1) * 512],
                            in0=wps[0:HD, :], in1=rbcs, op=mybir.AluOpType.mult)

        # ---------------- phase 3: output projection ----------------
        with pool("ph3", bufs=1) as ph3, \
             pool("ph3s", bufs=3) as ph3s, \
             pool("ps3", bufs=4, space="PSUM") as ps3:
            wo = ph3.tile([P, ND, D], bf16)
            nc.gpsimd.dma_start(
                out=wo, in_=ap["WoT"].rearrange("(g p) d -> p g d", p=P))
            for it in range(NT):
                ost = ph3s.tile([P, D], f32, tag="ost")
                for c in range(2):
                    ps = ps3.tile([P, 512], f32, tag="ps")
                    for g in range(ND):
                        nc.tensor.matmul(
                            ps, wvT[:, g, it * P:(it + 1) * P],
                            wo[:, g, c * 512:(c + 1) * 512],
                            start=(g == 0), stop=(g == ND - 1))
                    nc.vector.tensor_tensor(
                        out=ost[:, c * 512:(c + 1) * 512], in0=ps,
                        in1=bob[:, c * 512:(c + 1) * 512], op=ADD)
                nc.sync.dma_start(
                    out=ap["out"][it * P:(it + 1) * P, :], in_=ost)


def _build():
    if "nc" in _CACHE:
        return _CACHE["nc"]
    nc = bacc.Bacc("TRN2", target_bir_lowering=False, debug=False,
                   enable_asserts=False, num_devices=N_CORES)
    ap = {}
    for name, shape in (("x", [D, T]), ("WqT", [D, D]), ("WkT", [D, D]),
                        ("WvT", [D, D]), ("WoT", [D, D]), ("bq", [D]),
                        ("bv", [D]), ("bo", [D]), ("maskdT", [P, P])):
        ap[name] = nc.dram_tensor(name, shape, f32, kind="ExternalInput").ap()
    ap["out"] = nc.dram_tensor("out", [T, D], f32, kind="ExternalOutput").ap()
    ap["qk"] = nc.dram_tensor("qk", [H, T, T], f32, kind="ExternalOutput").ap()

    with tile.TileContext(nc) as tc:
        _emit(nc, tc, ap)
    nc.compile()
    nc.m = get_hw_module(nc.m)
    _CACHE["nc"] = nc
    return nc


def kernel(x, mask, Wq, bq, Wk, Wv, bv, Wo, bo):
    nc = _build()
    x = np.ascontiguousarray(x, dtype=np.float32)
    base = {
        "WqT": np.ascontiguousarray(Wq.T, dtype=np.float32),
        "WkT": np.ascontiguousarray(Wk.T, dtype=np.float32),
        "WvT": np.ascontiguousarray(Wv.T, dtype=np.float32),
        "WoT": np.ascontiguousarray(Wo.T, dtype=np.float32),
        "bq": np.ascontiguousarray(bq, dtype=np.float32),
        "bv": np.ascontiguousarray(bv, dtype=np.float32),
        "bo": np.ascontiguousarray(bo, dtype=np.float32),
        "maskdT": np.ascontiguousarray(mask[:P, :P].T, dtype=np.float32),
    }
    in_maps = [dict(base, x=np.ascontiguousarray(x[c].T)) for c in range(B)]

    res = bass_utils.run_bass_kernel_spmd(
        nc, in_maps, core_ids=list(range(N_CORES)), trace=TRACE)
    if TRACE:
        _CACHE["last_results"] = res

    out = np.stack([res.results[c]["out"] for c in range(B)])
    qk = np.stack([res.results[c]["qk"] for c in range(B)])
    triu = np.triu(np.ones((T, T), dtype=bool), k=1)
    qk[:, :, triu] = -np.inf
    return out, qk


# revision 16
# speedup vs baseline: 1.3190x; 1.1406x over previous
"""Self-contained Trainium2 Bass kernel for nn_MultiHeadAttention_50800873177468.

B=8, T=1024, D=1024, H=16 causal MHA (Whisper-style). Data-parallel over
batch: core c computes batch c. Returns (out, qk) like the reference.

Per-core dataflow (all matmuls f32r or bf16, fp32 PSUM accumulate):
  phase 1: qT = Wq@xT + bq, kT = 0.125*(Wk@xT), v = x@WvT + bv
  phase 2: per (head, tq-tile): s = qT.T@kT (K=64), +mask on diag block,
           qk out <- s (ACT copy), w = exp(s) bf16 + row-sum (ACT accum),
           w *= 1/sum (DVE), wT = PE-transpose(w), wv^T += v.T@wT
  phase 3: out = wv@WoT + bo
Host: input transposes, -inf fill of the qk upper triangle, stacking.
"""
import os
import sys
import types

for _p in ("/opt/trn_rl_repo", "/root/.axon_site/_ro/trn_rl_repo"):
    if os.path.isdir(_p) and _p not in sys.path:
        sys.path.append(_p)

import numpy as np

# NTFF profile hook shim (missing antenv.axon_hooks in this image)
if 'antenv.axon_hooks' not in sys.modules:
    _m = types.ModuleType('antenv.axon_hooks')
    _h = [None]
    _m.get_axon_ntff_profile_hook = lambda: _h[0]
    _m.set_axon_ntff_profile_hook = lambda h: _h.__setitem__(0, h)
    sys.modules['antenv.axon_hooks'] = _m
    try:
        from trn_agent_boot.trn_boot import _ntff_profile_via_ctypes
        _m.set_axon_ntff_profile_hook(
            _ntff_profile_via_ctypes('/opt/axon/libaxon_pjrt.so'))
    except Exception:
        pass

import concourse.bass as bass
import concourse.tile as tile
from concourse import bacc, mybir
from concourse import bass_utils
from concourse.bass_interp import get_hw_module

f32 = mybir.dt.float32
f32r = mybir.dt.float32r
bf16 = mybir.dt.bfloat16
EXP = mybir.ActivationFunctionType.Exp
ADD = mybir.AluOpType.add

N_CORES = 8
B, T, D, H = 8, 1024, 1024, 16
HD = D // H            # 64
P = 128
NT = T // P            # 8 tq tiles
ND = D // P            # 8 d tiles
S2 = 0.125             # (hd ** -0.25) ** 2, exact in fp32

TRACE = bool(os.environ.get("BASS_KERNEL_TRACE"))
_CACHE = {}


def _emit(nc, tc, ap):
    ctx_pools = []

    def pool(name, **kw):
        p = tc.tile_pool(name=name, **kw)
        return p

    with pool("persist", bufs=1) as persist:
        # small persistent operands
        bqs = persist.tile([P, ND], f32)          # bq as per-partition cols
        nc.sync.dma_start(out=bqs, in_=ap["bq"].rearrange("(g p) -> p g", p=P))
        maskdT = persist.tile([P, P], f32)        # transposed diag mask block
        nc.sync.dma_start(out=maskdT, in_=ap["maskdT"])
        ones_j = persist.tile([1, HD], bf16)      # ones row (broadcast lhsT)
        nc.vector.memset(ones_j, 1.0)
        bvb = persist.tile([P, D], bf16)          # bv broadcast along partitions
        nc.gpsimd.dma_start(out=bvb, in_=bass.AP(
            tensor=ap["bv"].tensor, offset=ap["bv"].offset,
            ap=[[0, P], [1, D]]))
        bob = persist.tile([P, D], f32)           # bo broadcast
        nc.gpsimd.dma_start(out=bob, in_=bass.AP(
            tensor=ap["bo"].tensor, offset=ap["bo"].offset,
            ap=[[0, P], [1, D]]))

        with pool("qkv", bufs=1) as qkv:
            qT = qkv.tile([P, ND, T], f32r)       # q^T  [dout, t]
            kT = qkv.tile([P, ND, T], f32r)       # k^T * 0.125
            qTb = qkv.tile([P, ND, T], bf16)      # bf16 copies (softmax path)
            kTb = qkv.tile([P, ND, T], bf16)
            vA = qkv.tile([P, NT, H, HD + 1], bf16)  # v | ones column
            wvT = qkv.tile([P, ND, T], bf16)      # (w@v)^T  [dj, tq]

            # ---------------- phase 1: projections ----------------
            # kk-outer with 8 live PSUM banks per half; weights stream in
            # per-k-tile chunks so DMA pipelines under the matmuls and the
            # PE never stalls at matrix boundaries (HAM stays warm).
            with pool("ph1", bufs=1) as ph1, \
                 pool("ph1w", bufs=9) as ph1w, \
                 pool("ps1", bufs=8, space="PSUM") as ps1:
                xT = ph1.tile([P, ND, T], f32r)
                nc.vector.memset(vA[:, :, :, HD:HD + 1], 1.0)

                for wname in ("WqT", "WkT", "WvT"):
                    wtks = []
                    for kk in range(ND):
                        if wname == "WqT":
                            nc.gpsimd.dma_start(
                                out=xT[:, kk, :],
                                in_=ap["x"][kk * P:(kk + 1) * P, :])
                        wtk = ph1w.tile([P, D], f32r, tag="w",
                                        name=f"w_{wname}_{kk}")
                        nc.gpsimd.dma_start(
                            out=wtk, in_=ap[wname][kk * P:(kk + 1) * P, :])
                        wtks.append(wtk)
                    for half in range(2):
                        pss = [ps1.tile([P, 512], f32, tag="ps",
                                        name=f"ps_{wname}_{half}_{j}")
                               for j in range(8)]
                        for kk in range(ND):
                            wtk = wtks[kk]
                            for j in range(8):
                                g, c = 4 * half + j // 2, j % 2
                                if wname == "WvT":
                                    lhsT = xT[:, kk, g * P:(g + 1) * P]
                                    rhs = wtk[:, c * 512:(c + 1) * 512]
                                else:
                                    lhsT = wtk[:, g * P:(g + 1) * P]
                                    rhs = xT[:, kk, c * 512:(c + 1) * 512]
                                nc.tensor.matmul(
                                    pss[j], lhsT, rhs,
                                    start=(kk == 0), stop=(kk == ND - 1))
                        for j in range(8):
                            g, c = 4 * half + j // 2, j % 2
                            ps = pss[j]
                            if wname == "WqT":
                                o = qT[:, g, c * 512:(c + 1) * 512]
                                nc.vector.tensor_scalar_add(o, ps, bqs[:, g:g + 1])
                                nc.gpsimd.tensor_copy(
                                    qTb[:, g, c * 512:(c + 1) * 512], o)
                            elif wname == "WkT":
                                o = kT[:, g, c * 512:(c + 1) * 512]
                                nc.vector.tensor_scalar_mul(o, ps, S2)
                                nc.gpsimd.tensor_copy(
                                    kTb[:, g, c * 512:(c + 1) * 512], o)
                            else:
                                tt = g  # [t-tile, d-chunk] output for v
                                for hh in range(8 * c, 8 * c + 8):
                                    nc.vector.tensor_tensor(
                                        out=vA[:, tt, hh, 0:HD],
                                        in0=ps[:, (hh - 8 * c) * HD:
                                               (hh - 8 * c + 1) * HD],
                                        in1=bvb[:, hh * HD:(hh + 1) * HD],
                                        op=ADD)

            # ---------------- phase 2: attention ----------------
            # Two score passes, no PE transposes:
            #  pass A [tq,tk] feeds the qk output; pass B [tk,tq] feeds
            #  exp -> w^T (bf16). Row sums via ones-matmul on w^T (l̃ lands
            #  [1,tq] in PSUM), normalization as (ones64 x 1/l̃) rank-1
            #  PSUM tile multiplied into the wv PSUM during copy-out.
            with pool("ph2s", bufs=4) as ph2s, \
                 pool("ph2w", bufs=2) as ph2w, \
                 pool("psA", bufs=2, space="PSUM") as psA, \
                 pool("psB", bufs=2, space="PSUM") as psB, \
                 pool("psW", bufs=2, space="PSUM") as psW:
                for h in range(H):
                    g, ro = h // 2, (h % 2) * HD
                    # --- pass A: scores in [tq, tk] for the qk output ---
                    for i in range(NT):
                        span = (i + 1) * P
                        qkst = ph2s.tile([P, T], f32, tag="qkst")
                        for c0 in range(0, span, 512):
                            c1 = min(c0 + 512, span)
                            sa = psA.tile([P, 512], f32, tag="sA")
                            nc.tensor.matmul(
                                sa[:, :c1 - c0],
                                qT[ro:ro + HD, g, i * P:(i + 1) * P],
                                kT[ro:ro + HD, g, c0:c1],
                                start=True, stop=True)
                            if (i * H + h) % 3 == 2:
                                nc.scalar.copy(qkst[:, c0:c1], sa[:, :c1 - c0])
                            else:
                                nc.vector.tensor_copy(qkst[:, c0:c1],
                                                      sa[:, :c1 - c0])
                        nc.sync.dma_start(
                            out=ap["qk"][h, i * P:(i + 1) * P, 0:span],
                            in_=qkst[:, :span])
                    # --- pass B: scores in [tk, tq] -> exp -> w^T bf16 ---
                    wTu = ph2w.tile([P, NT, T], bf16, tag="wTu")
                    for t in range(NT):
                        tq0 = t * P
                        width = T - tq0
                        sb = psB.tile([P, T], f32, tag="sB")
                        for c0 in range(tq0, T, 512):
                            c1 = min(c0 + 512, T)
                            nc.tensor.matmul(
                                sb[:, c0 - tq0:c1 - tq0],
                                kTb[ro:ro + HD, g, tq0:(t + 1) * P],
                                qTb[ro:ro + HD, g, c0:c1],
                                start=True, stop=True)
                        nc.vector.tensor_tensor(
                            out=sb[:, 0:P], in0=sb[:, 0:P],
                            in1=maskdT, op=ADD)
                        nc.scalar.activation(
                            wTu[:, t, tq0:T], sb[:, :width], EXP)
                    # --- wv (+ fused l̃ row via the ones column of vA) ---
                    def emit_wv(h, wTu):
                        g, ro = h // 2, (h % 2) * HD
                        for c in range(2):
                            wps = psW.tile([HD + 1, 512], f32, tag="wv",
                                           name=f"wv{h}_{c}")
                            nmm = 4 * c + 4
                            for t in range(nmm):
                                cs = max(c * 512, t * P)
                                nc.tensor.matmul(
                                    wps[:, cs - c * 512:512],
                                    vA[:, t, h, :],
                                    wTu[:, t, cs:(c + 1) * 512],
                                    start=(t == 0), stop=(t == nmm - 1))
                            lt = ph2s.tile([1, 512], f32, tag="lt",
                                           name=f"lt{h}_{c}")
                            nc.vector.tensor_copy(lt, wps[HD:HD + 1, :])
                            rt = ph2s.tile([1, 512], f32, tag="rt",
                                           name=f"rt{h}_{c}")
                            nc.vector.reciprocal_approx_fast(rt, lt)
                            rtb = ph2s.tile([1, 512], bf16, tag="rtb",
                                            name=f"rtb{h}_{c}")
                            nc.vector.tensor_copy(rtb, rt)
                            rbc = psA.tile([P, 512], f32, tag="sA",
                                           name=f"rbc{h}_{c}")[0:HD, :]
                            nc.tensor.matmul(rbc, ones_j, rtb,
                                             start=True, stop=True)
                            rbcs = ph2s.tile([HD, 512], bf16, tag="rbcs",
                                             name=f"rbcs{h}_{c}")
                            nc.scalar.copy(rbcs, rbc)
                            nc.vector.tensor_tensor(
                                out=wvT[ro:ro + HD, g,
                                        c * 512:(c + 1) * 512],
                                in0=wps[0:HD, :], in1=rbcs,
                                op=mybir.AluOpType.mult)
                    if h > 0:
                        emit_wv(h - 1, prev_wTu)
                    prev_wTu = wTu
                emit_wv(H - 1, prev_wTu)

        # ---------------- phase 3: output projection ----------------
        with pool("ph3", bufs=1) as ph3, \
             pool("ph3s", bufs=3) as ph3s, \
             pool("ps3", bufs=4, space="PSUM") as ps3:
            wo = ph3.tile([P, ND, D], bf16)
            for kk in range(ND):
                nc.gpsimd.dma_start(
                    out=wo[:, kk, :], in_=ap["WoT"][kk * P:(kk + 1) * P, :])
            for it in range(NT):
                ost = ph3s.tile([P, D], f32, tag="ost")
                for c in range(2):
                    ps = ps3.tile([P, 512], f32, tag="ps")
                    for g in range(ND):
                        nc.tensor.matmul(
                            ps, wvT[:, g, it * P:(it + 1) * P],
                            wo[:, g, c * 512:(c + 1) * 512],
                            start=(g == 0), stop=(g == ND - 1))
                    nc.vector.tensor_tensor(
                        out=ost[:, c * 512:(c + 1) * 512], in0=ps,
                        in1=bob[:, c * 512:(c + 1) * 512], op=ADD)
                nc.sync.dma_start(
                    out=ap["out"][it * P:(it + 1) * P, :], in_=ost)


def _build():
    if "nc" in _CACHE:
        return _CACHE["nc"]
    nc = bacc.Bacc("TRN2", target_bir_lowering=False, debug=False,
                   enable_asserts=False, num_devices=N_CORES)
    ap = {}
    for name, shape in (("x", [D, T]), ("WqT", [D, D]), ("WkT", [D, D]),
                        ("WvT", [D, D]), ("WoT", [D, D]), ("bq", [D]),
                        ("bv", [D]), ("bo", [D]), ("maskdT", [P, P])):
        ap[name] = nc.dram_tensor(name, shape, f32, kind="ExternalInput").ap()
    ap["out"] = nc.dram_tensor("out", [T, D], f32, kind="ExternalOutput").ap()
    ap["qk"] = nc.dram_tensor("qk", [H, T, T], f32, kind="ExternalOutput").ap()

    with tile.TileContext(nc) as tc:
        _emit(nc, tc, ap)
    nc.compile()
    nc.m = get_hw_module(nc.m)
    _CACHE["nc"] = nc
    return nc


def kernel(x, mask, Wq, bq, Wk, Wv, bv, Wo, bo):
    nc = _build()
    x = np.ascontiguousarray(x, dtype=np.float32)
    base = {
        "WqT": np.ascontiguousarray(Wq.T, dtype=np.float32),
        "WkT": np.ascontiguousarray(Wk.T, dtype=np.float32),
        "WvT": np.ascontiguousarray(Wv.T, dtype=np.float32),
        "WoT": np.ascontiguousarray(Wo.T, dtype=np.float32),
        "bq": np.ascontiguousarray(bq, dtype=np.float32),
        "bv": np.ascontiguousarray(bv, dtype=np.float32),
        "bo": np.ascontiguousarray(bo, dtype=np.float32),
        "maskdT": np.ascontiguousarray(mask[:P, :P].T, dtype=np.float32),
    }
    in_maps = [dict(base, x=np.ascontiguousarray(x[c].T)) for c in range(B)]

    res = bass_utils.run_bass_kernel_spmd(
        nc, in_maps, core_ids=list(range(N_CORES)), trace=TRACE)
    if TRACE:
        _CACHE["last_results"] = res

    out = np.stack([res.results[c]["out"] for c in range(B)])
    qk = np.stack([res.results[c]["qk"] for c in range(B)])
    triu = np.triu(np.ones((T, T), dtype=bool), k=1)
    qk[:, :, triu] = -np.inf
    return out, qk
